# revision 4
# baseline (speedup 1.0000x reference)
"""Trainium2 Bass kernel for GAT layer (gnn_message_passing).

Strategy: shard edges by destination-node range across 8 cores (dst is
repeat(arange(N), 8), so a dst range == a contiguous edge range). Each core:
  Phase A: builds full-node gather tables in its local DRAM:
      h1  = leaky_relu(atom @ W1.T + b1)            [N, 128]
      P2  = atom @ W2a.T + b2                        [N, 128]
    (replicated compute -- avoids collectives entirely), plus its own node
    slice of h1 kept resident in SBUF and q = h1 @ w_d per local node.
  Phase B: for each supertile of 1024 edges (= 128 dst nodes, 8 edges each,
    node-major layout: partition p holds node p's 8 edges along free axis):
      - bond part of nn via PE matmuls from host-pretransposed bond chunks
      - indirect-gather P2[src] with CCE-add accumulating onto the bond part
      - logits l_e = prelu(q[dst] + prelu(nn) . w_e + bw); segment softmax
        over the 8 free-axis slots (no max-subtraction needed; logits are
        O(1) so exp is safe in fp32)
      - weighted sum of gathered h1[src], then attend (Wa) + elu + GRU cell
        fused per 128-node tile; write final slice.
Outputs (final, h1) are concatenated from the per-core node slices.
"""

import os
import sys

for _p in ("/opt/trn_rl_repo", "/root/.axon_site/_ro/trn_rl_repo"):
    if os.path.isdir(_p) and _p not in sys.path:
        sys.path.insert(0, _p)

import numpy as np

LEAKY = 0.2
ELU_A = 1.0

# problem shape (hardcoded per contract)
N_NODES = 100000
DEG = 8
D_IN = 128
D_BOND = 64
D_OUT = 128
N_CORES = 8


class _Cfg:
    def __init__(self, n_nodes=N_NODES, n_cores=N_CORES, sim_safe=False):
        self.n_nodes = n_nodes
        self.n_cores = n_cores
        self.sim_safe = sim_safe  # emit DVE leaky-relu instead of ACT Prelu (CoreSim lacks Prelu)
        self.nc_nodes = n_nodes // n_cores          # nodes per core
        self.nc_edges = self.nc_nodes * DEG         # edges per core
        self.S = -(-self.nc_nodes // 128)           # supertiles per core
        self.npad = self.S * 128                    # padded nodes per core
        self.epad = self.npad * DEG                 # padded edges per core
        self.t1 = -(-n_nodes // 128)                # phase-A1 tiles
        self.ntab = self.t1 * 128                   # padded table rows


_cache = {}


def _build(cfg):
    import concourse.bass as bass
    import concourse.mybir as mybir
    import concourse.tile as tile
    from concourse import bacc
    from concourse.masks import make_identity

    f32 = mybir.dt.float32
    i32 = mybir.dt.int32
    AF = mybir.ActivationFunctionType
    OP = mybir.AluOpType

    S, T1, NTAB, NPAD = cfg.S, cfg.t1, cfg.ntab, cfg.npad

    nc = bacc.Bacc("TRN2")

    # ---- inputs
    atom_pad = nc.dram_tensor("atom_pad", [NTAB, D_IN], f32, kind="ExternalInput")
    atom_my = nc.dram_tensor("atom_my", [NPAD, D_IN], f32, kind="ExternalInput")
    src_nat = nc.dram_tensor("src_nat", [S, 128, DEG], i32, kind="ExternalInput")
    bondT_in = nc.dram_tensor("bondT_in", [S, D_BOND, 1024], f32, kind="ExternalInput")
    W1T_in = nc.dram_tensor("W1T_in", [D_IN, D_OUT], f32, kind="ExternalInput")
    W2aT_in = nc.dram_tensor("W2aT_in", [D_IN, D_OUT], f32, kind="ExternalInput")
    W2bT_in = nc.dram_tensor("W2bT_in", [D_BOND, D_OUT], f32, kind="ExternalInput")
    WaT_in = nc.dram_tensor("WaT_in", [D_OUT, D_OUT], f32, kind="ExternalInput")
    WihT_in = nc.dram_tensor("WihT_in", [D_OUT, 3 * D_OUT], f32, kind="ExternalInput")
    WhrzT_in = nc.dram_tensor("WhrzT_in", [D_OUT, 2 * D_OUT], f32, kind="ExternalInput")
    WhnT_in = nc.dram_tensor("WhnT_in", [D_OUT, D_OUT], f32, kind="ExternalInput")
    webc_in = nc.dram_tensor("webc_in", [128, D_OUT], f32, kind="ExternalInput")
    wdbc_in = nc.dram_tensor("wdbc_in", [128, D_OUT], f32, kind="ExternalInput")
    b1_in = nc.dram_tensor("b1_in", [1, D_OUT], f32, kind="ExternalInput")
    b2_in = nc.dram_tensor("b2_in", [1, D_OUT], f32, kind="ExternalInput")
    ba_in = nc.dram_tensor("ba_in", [1, D_OUT], f32, kind="ExternalInput")
    bg_in = nc.dram_tensor("bg_in", [1, 3 * D_OUT], f32, kind="ExternalInput")
    bhn_in = nc.dram_tensor("bhn_in", [1, D_OUT], f32, kind="ExternalInput")
    ones_in = nc.dram_tensor("ones_in", [1, 128], f32, kind="ExternalInput")

    # ---- outputs
    final_out = nc.dram_tensor("final_out", [NPAD, D_OUT], f32, kind="ExternalOutput")
    h1_out = nc.dram_tensor("h1_out", [NPAD, D_OUT], f32, kind="ExternalOutput")

    # ---- internal gather tables
    h1_tab = nc.dram_tensor("h1_tab", [NTAB, D_OUT], f32)
    p2_tab = nc.dram_tensor("p2_tab", [NTAB, D_OUT], f32)

    bw_scalar = None  # set via host (bias immediate); use placeholder replaced below

    with tile.TileContext(nc) as tc:
        with (
            tc.tile_pool(name="consts", bufs=1) as cpool,
            tc.tile_pool(name="resident", bufs=1) as rpool,
        ):
            W1T = cpool.tile([D_IN, D_OUT], f32)
            nc.sync.dma_start(W1T[:], W1T_in[:])
            W2aT = cpool.tile([D_IN, D_OUT], f32)
            nc.sync.dma_start(W2aT[:], W2aT_in[:])
            W2bT = cpool.tile([D_BOND, D_OUT], f32)
            nc.sync.dma_start(W2bT[:], W2bT_in[:])
            WaT = cpool.tile([D_OUT, D_OUT], f32)
            nc.sync.dma_start(WaT[:], WaT_in[:])
            WihT = cpool.tile([D_OUT, 3 * D_OUT], f32)
            nc.sync.dma_start(WihT[:], WihT_in[:])
            WhrzT = cpool.tile([D_OUT, 2 * D_OUT], f32)
            nc.sync.dma_start(WhrzT[:], WhrzT_in[:])
            WhnT = cpool.tile([D_OUT, D_OUT], f32)
            nc.sync.dma_start(WhnT[:], WhnT_in[:])
            webc = cpool.tile([128, D_OUT], f32)
            nc.sync.dma_start(webc[:], webc_in[:])
            wdbc = cpool.tile([128, D_OUT], f32)
            nc.sync.dma_start(wdbc[:], wdbc_in[:])
            b1r = cpool.tile([1, D_OUT], f32)
            nc.sync.dma_start(b1r[:], b1_in[:])
            b2r = cpool.tile([1, D_OUT], f32)
            nc.sync.dma_start(b2r[:], b2_in[:])
            bar = cpool.tile([1, D_OUT], f32)
            nc.sync.dma_start(bar[:], ba_in[:])
            bgr = cpool.tile([1, 3 * D_OUT], f32)
            nc.sync.dma_start(bgr[:], bg_in[:])
            bhnr = cpool.tile([1, D_OUT], f32)
            nc.sync.dma_start(bhnr[:], bhn_in[:])
            onesr = cpool.tile([1, 128], f32)
            nc.sync.dma_start(onesr[:], ones_in[:])
            idn = cpool.tile([128, 128], f32)
            make_identity(nc, idn[:])

            h1_my = rpool.tile([128, S, D_OUT], f32)
            q_my = rpool.tile([128, S], f32)

            def prelu(out_ap, in_ap, shape, pool, tag):
                if cfg.sim_safe:
                    tmp = pool.tile(shape, f32, tag=tag)
                    nc.vector.tensor_scalar(tmp[:], in_ap, LEAKY, None, OP.mult)
                    nc.vector.tensor_tensor(out=out_ap, in0=in_ap, in1=tmp[:], op=OP.max)
                else:
                    nc.scalar.activation(out_ap, in_ap, AF.Prelu, alpha=LEAKY)

            # ================= Phase A =================
            with (
                tc.tile_pool(name="pha", bufs=3) as pa,
                tc.tile_pool(name="pha_ps", bufs=2, space="PSUM") as pap,
            ):
                def node_tile(src_dram, row0, want_p2, a2_idx):
                    at = pa.tile([128, D_IN], f32, tag="at")
                    nc.sync.dma_start(at[:], src_dram[row0:row0 + 128, :])
                    atT_ps = pap.tile([128, 128], f32, tag="atT_ps")
                    nc.tensor.transpose(atT_ps[:], at[:], idn[:])
                    atT = pa.tile([128, 128], f32, tag="atT")
                    nc.scalar.activation(atT[:], atT_ps[:], AF.Copy)
                    h1_ps = pap.tile([128, D_OUT], f32, tag="h1_ps")
                    nc.tensor.matmul(h1_ps[:], lhsT=atT[:], rhs=W1T[:], start=True, stop=False)
                    nc.tensor.matmul(h1_ps[:], lhsT=onesr[:], rhs=b1r[:], start=False, stop=True)
                    if want_p2:
                        p2_ps = pap.tile([128, D_OUT], f32, tag="p2_ps")
                        nc.tensor.matmul(p2_ps[:], lhsT=atT[:], rhs=W2aT[:], start=True, stop=False)
                        nc.tensor.matmul(p2_ps[:], lhsT=onesr[:], rhs=b2r[:], start=False, stop=True)
                        p2_sb = pa.tile([128, D_OUT], f32, tag="p2_sb")
                        nc.vector.tensor_copy(p2_sb[:], p2_ps[:])
                        nc.sync.dma_start(p2_tab[row0:row0 + 128, :], p2_sb[:])
                    if a2_idx is None:
                        h1_sb = pa.tile([128, D_OUT], f32, tag="h1_sb")
                        prelu(h1_sb[:], h1_ps[:], [128, D_OUT], pa, "lr_a")
                        nc.sync.dma_start(h1_tab[row0:row0 + 128, :], h1_sb[:])
                    else:
                        t = a2_idx
                        prelu(h1_my[:, t, :], h1_ps[:], [128, D_OUT], pa, "lr_a")
                        nc.sync.dma_start(h1_out[row0:row0 + 128, :], h1_my[:, t, :])
                        qscr = pa.tile([128, D_OUT], f32, tag="qscr")
                        nc.vector.tensor_tensor(out=qscr[:], in0=h1_my[:, t, :], in1=wdbc[:], op=OP.mult)
                        nc.vector.tensor_reduce(out=q_my[:, t:t + 1], in_=qscr[:], axis=mybir.AxisListType.X, op=OP.add)

                for t in range(T1):
                    node_tile(atom_pad, t * 128, True, None)
                for t in range(S):
                    node_tile(atom_my, t * 128, False, t)

            tc.strict_bb_all_engine_barrier()

            # ================= Phase B =================
            with (
                tc.tile_pool(name="phb", bufs=2) as pb,
                tc.tile_pool(name="phb_nn_ps", bufs=1, space="PSUM") as pbn,
                tc.tile_pool(name="phb_ps", bufs=1, space="PSUM") as pbp,
            ):
                for s in range(S):
                    idx_t = pb.tile([128, DEG], i32, tag="idx")
                    nc.sync.dma_start(idx_t[:], src_nat[s])
                    bT = pb.tile([D_BOND, DEG, 128], f32, tag="bT")
                    nc.sync.dma_start(
                        bT[:].rearrange("p a b -> p (a b)"), bondT_in[s]
                    )

                    nn_ps = pbn.tile([128, DEG * 128], f32, tag="nn_ps")
                    for q in range(DEG):
                        nc.tensor.matmul(
                            nn_ps[:, q * 128:(q + 1) * 128],
                            lhsT=bT[:, q, :], rhs=W2bT[:], start=True, stop=True)
                    pre = pb.tile([128, DEG * 128], f32, tag="pre")
                    nc.scalar.activation(pre[:], nn_ps[:], AF.Copy)
                    for q in range(DEG):
                        nc.gpsimd.indirect_dma_start(
                            out=pre[:, q * 128:(q + 1) * 128], out_offset=None,
                            in_=p2_tab[:],
                            in_offset=bass.IndirectOffsetOnAxis(ap=idx_t[:, q:q + 1], axis=0),
                            compute_op=OP.add)
                    h1g = pb.tile([128, DEG, 128], f32, tag="h1g")
                    for q in range(DEG):
                        nc.gpsimd.indirect_dma_start(
                            out=h1g[:, q, :], out_offset=None,
                            in_=h1_tab[:],
                            in_offset=bass.IndirectOffsetOnAxis(ap=idx_t[:, q:q + 1], axis=0))

                    nnact = pb.tile([128, DEG * 128], f32, tag="nnact")
                    prelu(nnact[:], pre[:], [128, DEG * 128], pb, "lr_b")
                    nnw = pb.tile([128, DEG, 128], f32, tag="nnw")
                    nc.vector.tensor_tensor(
                        out=nnw[:], in0=nnact[:].rearrange("p (a b) -> p a b", a=DEG),
                        in1=webc[:, None, :].to_broadcast([128, DEG, 128]), op=OP.mult)
                    pl = pb.tile([128, DEG], f32, tag="pl")
                    nc.vector.tensor_reduce(out=pl[:], in_=nnw[:], axis=mybir.AxisListType.X, op=OP.add)

                    z8 = pb.tile([128, DEG], f32, tag="z8")
                    nc.vector.tensor_scalar(z8[:], pl[:], q_my[:, s:s + 1], _BW[0], OP.add, OP.add)
                    l8 = pb.tile([128, DEG], f32, tag="l8")
                    prelu(l8[:], z8[:], [128, DEG], pb, "lr_c")
                    w8 = pb.tile([128, DEG], f32, tag="w8")
                    dsum = pb.tile([128, 1], f32, tag="dsum")
                    nc.scalar.activation(w8[:], l8[:], AF.Exp, accum_out=dsum[:])
                    rd = pb.tile([128, 1], f32, tag="rd")
                    nc.vector.reciprocal(rd[:], dsum[:])
                    sc8 = pb.tile([128, DEG], f32, tag="sc8")
                    nc.vector.tensor_scalar(sc8[:], w8[:], rd[:], None, OP.mult)

                    wh = pb.tile([128, DEG, 128], f32, tag="wh")
                    nc.vector.tensor_tensor(
                        out=wh[:], in0=h1g[:],
                        in1=sc8[:, :, None].to_broadcast([128, DEG, 128]), op=OP.mult)
                    traw = pb.tile([128, 128], f32, tag="traw")
                    nc.vector.tensor_reduce(
                        out=traw[:], in_=wh[:].rearrange("p a b -> p b a"),
                        axis=mybir.AxisListType.X, op=OP.add)

                    trT_ps = pbp.tile([128, 128], f32, tag="tps")
                    nc.tensor.transpose(trT_ps[:], traw[:], idn[:])
                    trT = pb.tile([128, 128], f32, tag="trT")
                    nc.scalar.activation(trT[:], trT_ps[:], AF.Copy)
                    wa_ps = pbp.tile([128, D_OUT], f32, tag="wa_ps")
                    nc.tensor.matmul(wa_ps[:], lhsT=trT[:], rhs=WaT[:], start=True, stop=False)
                    nc.tensor.matmul(wa_ps[:], lhsT=onesr[:], rhs=bar[:], start=False, stop=True)

                    # elu(x) + 1 = relu(x) + exp(min(x, 0)); the -1 is folded in bg
                    m0 = pb.tile([128, 128], f32, tag="m0")
                    nc.vector.tensor_scalar(m0[:], wa_ps[:], 0.0, None, OP.min)
                    e0 = pb.tile([128, 128], f32, tag="e0")
                    nc.scalar.activation(e0[:], m0[:], AF.Exp)
                    r0 = pb.tile([128, 128], f32, tag="r0")
                    nc.vector.tensor_scalar(r0[:], wa_ps[:], 0.0, None, OP.max)
                    ctxs = pb.tile([128, 128], f32, tag="ctxs")
                    nc.vector.tensor_tensor(out=ctxs[:], in0=r0[:], in1=e0[:], op=OP.add)

                    cT_ps = pbp.tile([128, 128], f32, tag="tps")
                    nc.tensor.transpose(cT_ps[:], ctxs[:], idn[:])
                    cT = pb.tile([128, 128], f32, tag="cT")
                    nc.scalar.activation(cT[:], cT_ps[:], AF.Copy)
                    hT_ps = pbp.tile([128, 128], f32, tag="tps")
                    nc.tensor.transpose(hT_ps[:], h1_my[:, s, :], idn[:])
                    hT = pb.tile([128, 128], f32, tag="hT")
                    nc.scalar.activation(hT[:], hT_ps[:], AF.Copy)

                    g_ps = pbp.tile([128, 3 * D_OUT], f32, tag="g_ps")
                    nc.tensor.matmul(g_ps[:], lhsT=cT[:], rhs=WihT[:], start=True, stop=False)
                    nc.tensor.matmul(g_ps[:, :2 * D_OUT], lhsT=hT[:], rhs=WhrzT[:], start=False, stop=False)
                    nc.tensor.matmul(g_ps[:], lhsT=onesr[:], rhs=bgr[:], start=False, stop=True)
                    hn_ps = pbp.tile([128, D_OUT], f32, tag="hn_ps")
                    nc.tensor.matmul(hn_ps[:], lhsT=hT[:], rhs=WhnT[:], start=True, stop=False)
                    nc.tensor.matmul(hn_ps[:], lhsT=onesr[:], rhs=bhnr[:], start=False, stop=True)

                    rz = pb.tile([128, 2 * D_OUT], f32, tag="rz")
                    nc.scalar.activation(rz[:], g_ps[:, :2 * D_OUT], AF.Sigmoid)
                    rhn = pb.tile([128, 128], f32, tag="rhn")
                    nc.vector.tensor_tensor(out=rhn[:], in0=hn_ps[:], in1=rz[:, :128], op=OP.mult)
                    npre = pb.tile([128, 128], f32, tag="npre")
                    nc.vector.tensor_tensor(out=npre[:], in0=rhn[:], in1=g_ps[:, 2 * D_OUT:], op=OP.add)
                    ngate = pb.tile([128, 128], f32, tag="ngate")
                    nc.scalar.activation(ngate[:], npre[:], AF.Tanh)
                    d1 = pb.tile([128, 128], f32, tag="d1")
                    nc.vector.tensor_tensor(out=d1[:], in0=h1_my[:, s, :], in1=ngate[:], op=OP.subtract)
                    d2 = pb.tile([128, 128], f32, tag="d2")
                    nc.vector.tensor_tensor(out=d2[:], in0=d1[:], in1=rz[:, 128:], op=OP.mult)
                    fin = pb.tile([128, 128], f32, tag="fin")
                    nc.vector.tensor_tensor(out=fin[:], in0=d2[:], in1=ngate[:], op=OP.add)
                    nc.sync.dma_start(final_out[s * 128:(s + 1) * 128, :], fin[:])

    nc.finalize()
    return nc


_BW = [0.0]  # bw bias immediate, set before _build


def _host_prep(inputs, cfg):
    atom = np.ascontiguousarray(inputs["atom_features"], dtype=np.float32)
    bond = np.ascontiguousarray(inputs["bond_feats"], dtype=np.float32)
    src = np.ascontiguousarray(inputs["src"], dtype=np.int32)
    W1 = inputs["W1"].astype(np.float32)
    b1 = inputs["b1"].astype(np.float32)
    W2 = inputs["W2"].astype(np.float32)
    b2 = inputs["b2"].astype(np.float32)
    Wa = inputs["Wa"].astype(np.float32)
    ba = inputs["ba"].astype(np.float32)
    Ww = inputs["Ww"].astype(np.float32)
    bw = inputs["bw"].astype(np.float32)
    W_ih = inputs["W_ih"].astype(np.float32)
    b_ih = inputs["b_ih"].astype(np.float32)
    W_hh = inputs["W_hh"].astype(np.float32)
    b_hh = inputs["b_hh"].astype(np.float32)

    n, C = cfg.n_nodes, cfg.n_cores
    ncn, nce = cfg.nc_nodes, cfg.nc_edges
    S, NPAD, EPAD, NTAB = cfg.S, cfg.npad, cfg.epad, cfg.ntab

    atom_pad = np.zeros((NTAB, D_IN), np.float32)
    atom_pad[:n] = atom

    w_d = Ww[0, :D_OUT].copy()
    w_e = Ww[0, D_OUT:].copy()
    _BW[0] = float(bw[0])

    shared = {
        "atom_pad": atom_pad,
        "W1T_in": np.ascontiguousarray(W1.T),
        "W2aT_in": np.ascontiguousarray(W2[:, :D_IN].T),
        "W2bT_in": np.ascontiguousarray(W2[:, D_IN:].T),
        "WaT_in": np.ascontiguousarray(Wa.T),
        "WihT_in": np.ascontiguousarray(W_ih.T),
        "WhrzT_in": np.ascontiguousarray(W_hh[:2 * D_OUT].T),
        "WhnT_in": np.ascontiguousarray(W_hh[2 * D_OUT:].T),
        "webc_in": np.ascontiguousarray(np.tile(w_e[None, :], (128, 1))),
        "wdbc_in": np.ascontiguousarray(np.tile(w_d[None, :], (128, 1))),
        "b1_in": b1[None, :].copy(),
        "b2_in": b2[None, :].copy(),
        "ba_in": ba[None, :].copy(),
        # gates psum bias: b_ih(-1 fold for ctx* = elu+1) + b_hh on r,z only
        "bg_in": (b_ih - W_ih.sum(axis=1)
                  + np.concatenate([b_hh[:2 * D_OUT], np.zeros(D_OUT, np.float32)])
                  )[None, :].astype(np.float32),
        "bhn_in": b_hh[2 * D_OUT:][None, :].copy(),
        "ones_in": np.ones((1, 128), np.float32),
    }

    in_maps = []
    for c in range(C):
        a_my = np.zeros((NPAD, D_IN), np.float32)
        a_my[:ncn] = atom[c * ncn:(c + 1) * ncn]
        src_pad = np.zeros(EPAD, np.int32)
        src_pad[:nce] = src[c * nce:(c + 1) * nce]
        bond_pad = np.zeros((EPAD, D_BOND), np.float32)
        bond_pad[:nce] = bond[c * nce:(c + 1) * nce]
        # src_nat[s, p, q] = src_pad[1024 s + 8 p + q]
        src_nat = np.ascontiguousarray(src_pad.reshape(S, 128, DEG))
        # bondT[s, f, q*128 + p] = bond_pad[1024 s + 8 p + q, f]
        bondT = np.ascontiguousarray(
            bond_pad.reshape(S, 128, DEG, D_BOND).transpose(0, 3, 2, 1)
        ).reshape(S, D_BOND, 1024)
        im = dict(shared)
        im["atom_my"] = a_my
        im["src_nat"] = src_nat
        im["bondT_in"] = bondT
        in_maps.append(im)
    return in_maps


def kernel(**inputs):
    from concourse.bass_utils import run_bass_kernel_spmd

    cfg = _Cfg()
    in_maps = _host_prep(inputs, cfg)
    key = (cfg.n_nodes, cfg.n_cores)
    if key not in _cache:
        _cache[key] = _build(cfg)
    nc = _cache[key]
    res = run_bass_kernel_spmd(nc, in_maps, list(range(cfg.n_cores)))
    ncn = cfg.nc_nodes
    final = np.concatenate([res.results[c]["final_out"][:ncn] for c in range(cfg.n_cores)], axis=0)
    h1 = np.concatenate([res.results[c]["h1_out"][:ncn] for c in range(cfg.n_cores)], axis=0)
    return final, h1


# revision 9
# speedup vs baseline: 9806.0490x; 9806.0490x over previous
"""Trainium2 Bass kernel for GAT layer (gnn_message_passing).

Sharding: edges by destination-node range across 8 cores (dst is
repeat(arange(N), 8), so a dst range == a contiguous edge range).

Per core:
  Phase A (replicated over full N; avoids collectives): builds gather tables
    h1 = leaky_relu(atom @ W1.T + b1), P2 = atom @ W2a.T + b2 in local DRAM.
    Works from a host-transposed atom (atomT), so the matmuls run
    weights-stationary with N=512 moving slabs; per-feature biases are folded
    into the PSUM-evacuation activations as per-partition bias columns; the
    [feat, node] results are transposed back on the PE. Also computes its own
    node slice of h1 (kept resident in SBUF) and q = h1 @ w_d.
  Phase B per supertile of 1024 edges (= 128 dst nodes x 8 edges, node-major:
    partition p holds node p's 8 edges on the free axis):
    - bond part of nn via PE matmuls from host-pretransposed bond chunks
    - indirect-gather P2[src] with CCE-add accumulating onto the bond part
    - logits prelu(q[dst] + prelu(nn) . w_e + bw); per-partition softmax over
      the 8 free slots (no max subtraction: logits are O(1), exp-safe in f32)
    - weighted sum of gathered h1[src]; attend (Wa) + elu + GRU fused per
      128-node tile (sigmoid via tanh half-angle so the whole kernel uses a
      single activation-table set). Writes the final slice.
Outputs (final, h1) are concatenated from the per-core node slices.
"""

import os
import sys

for _p in ("/opt/trn_rl_repo", "/root/.axon_site/_ro/trn_rl_repo"):
    if os.path.isdir(_p) and _p not in sys.path:
        sys.path.insert(0, _p)

import numpy as np

LEAKY = 0.2

N_NODES = 100000
DEG = 8
D_IN = 128
D_BOND = 64
D_OUT = 128
N_CORES = 8
SLAB = 512  # phase-A moving-operand width


class _Cfg:
    def __init__(self, n_nodes=N_NODES, n_cores=N_CORES, sim_safe=False):
        self.n_nodes = n_nodes
        self.n_cores = n_cores
        self.sim_safe = sim_safe
        self.nc_nodes = n_nodes // n_cores            # nodes per core
        self.nc_edges = self.nc_nodes * DEG           # edges per core
        self.S = -(-self.nc_nodes // 128)             # supertiles per core
        self.npad = self.S * 128                      # padded nodes (128) per core
        self.epad = self.npad * DEG
        self.ka2 = -(-self.npad // SLAB)              # phase-A2 slabs
        self.npad2 = self.ka2 * SLAB                  # padded nodes (512) per core
        self.tq = self.npad2 // 128
        self.k1 = -(-n_nodes // SLAB)                 # phase-A1 slabs
        self.ntab = self.k1 * SLAB                    # table rows


_cache = {}


def _build(cfg):
    import concourse.bass as bass
    import concourse.mybir as mybir
    import concourse.tile as tile
    from concourse import bacc
    from concourse.masks import make_identity

    f32 = mybir.dt.float32
    i32 = mybir.dt.int32
    AF = mybir.ActivationFunctionType
    OP = mybir.AluOpType
    X = mybir.AxisListType.X

    S, K1, KA2, NTAB, NPAD2, TQ = cfg.S, cfg.k1, cfg.ka2, cfg.ntab, cfg.npad2, cfg.tq

    nc = bacc.Bacc("TRN2")

    # ---- inputs
    atomT_pad = nc.dram_tensor("atomT_pad", [D_IN, NTAB], f32, kind="ExternalInput")
    atom_myT = nc.dram_tensor("atom_myT", [D_IN, NPAD2], f32, kind="ExternalInput")
    src_nat = nc.dram_tensor("src_nat", [S, 128, DEG], i32, kind="ExternalInput")
    bondT_in = nc.dram_tensor("bondT_in", [S, D_BOND, 1024], f32, kind="ExternalInput")
    W1T_in = nc.dram_tensor("W1T_in", [D_IN, D_OUT], f32, kind="ExternalInput")
    W2aT_in = nc.dram_tensor("W2aT_in", [D_IN, D_OUT], f32, kind="ExternalInput")
    W2bT_in = nc.dram_tensor("W2bT_in", [D_BOND, D_OUT], f32, kind="ExternalInput")
    WaT_in = nc.dram_tensor("WaT_in", [D_OUT, D_OUT], f32, kind="ExternalInput")
    WihT_in = nc.dram_tensor("WihT_in", [D_OUT, 3 * D_OUT], f32, kind="ExternalInput")
    WhrzT_in = nc.dram_tensor("WhrzT_in", [D_OUT, 2 * D_OUT], f32, kind="ExternalInput")
    WhnT_in = nc.dram_tensor("WhnT_in", [D_OUT, D_OUT], f32, kind="ExternalInput")
    webc_in = nc.dram_tensor("webc_in", [128, D_OUT], f32, kind="ExternalInput")
    wdbc_in = nc.dram_tensor("wdbc_in", [128, D_OUT], f32, kind="ExternalInput")
    b1c_in = nc.dram_tensor("b1c_in", [D_OUT, 1], f32, kind="ExternalInput")
    b2c_in = nc.dram_tensor("b2c_in", [D_OUT, 1], f32, kind="ExternalInput")
    ba_in = nc.dram_tensor("ba_in", [1, D_OUT], f32, kind="ExternalInput")
    bg_in = nc.dram_tensor("bg_in", [1, 3 * D_OUT], f32, kind="ExternalInput")
    bhn_in = nc.dram_tensor("bhn_in", [1, D_OUT], f32, kind="ExternalInput")
    ones_in = nc.dram_tensor("ones_in", [1, 128], f32, kind="ExternalInput")

    # ---- outputs
    final_out = nc.dram_tensor("final_out", [cfg.npad, D_OUT], f32, kind="ExternalOutput")
    h1_out = nc.dram_tensor("h1_out", [NPAD2, D_OUT], f32, kind="ExternalOutput")

    # ---- internal gather table: row n = [P2[n]+b2 | h1[n]] (1KB rows)
    gtab = nc.dram_tensor("gtab", [NTAB, 2 * D_OUT], f32)

    with tile.TileContext(nc) as tc:
        with (
            tc.tile_pool(name="consts", bufs=1) as cpool,
            tc.tile_pool(name="resident", bufs=1) as rpool,
        ):
            W1T = cpool.tile([D_IN, D_OUT], f32)
            nc.sync.dma_start(W1T[:], W1T_in[:])
            W2aT = cpool.tile([D_IN, D_OUT], f32)
            nc.sync.dma_start(W2aT[:], W2aT_in[:])
            W2bT = cpool.tile([D_BOND, D_OUT], f32)
            nc.sync.dma_start(W2bT[:], W2bT_in[:])
            WaT = cpool.tile([D_OUT, D_OUT], f32)
            nc.sync.dma_start(WaT[:], WaT_in[:])
            WihT = cpool.tile([D_OUT, 3 * D_OUT], f32)
            nc.sync.dma_start(WihT[:], WihT_in[:])
            WhrzT = cpool.tile([D_OUT, 2 * D_OUT], f32)
            nc.sync.dma_start(WhrzT[:], WhrzT_in[:])
            WhnT = cpool.tile([D_OUT, D_OUT], f32)
            nc.sync.dma_start(WhnT[:], WhnT_in[:])
            webc = cpool.tile([128, D_OUT], f32)
            nc.sync.dma_start(webc[:], webc_in[:])
            wdbc = cpool.tile([128, D_OUT], f32)
            nc.sync.dma_start(wdbc[:], wdbc_in[:])
            b1c = cpool.tile([D_OUT, 1], f32)
            nc.sync.dma_start(b1c[:], b1c_in[:])
            b2c = cpool.tile([D_OUT, 1], f32)
            nc.sync.dma_start(b2c[:], b2c_in[:])
            bar = cpool.tile([1, D_OUT], f32)
            nc.sync.dma_start(bar[:], ba_in[:])
            bgr = cpool.tile([1, 3 * D_OUT], f32)
            nc.sync.dma_start(bgr[:], bg_in[:])
            bhnr = cpool.tile([1, D_OUT], f32)
            nc.sync.dma_start(bhnr[:], bhn_in[:])
            onesr = cpool.tile([1, 128], f32)
            nc.sync.dma_start(onesr[:], ones_in[:])
            idn = cpool.tile([128, 128], f32)
            make_identity(nc, idn[:])

            h1_my = rpool.tile([128, TQ, D_OUT], f32)
            q_my = rpool.tile([128, TQ], f32)

            def prelu(out_ap, in_ap, shape, pool, tag, bias=0.0):
                if cfg.sim_safe:
                    if not isinstance(bias, float):
                        raise RuntimeError("sim_safe prelu with bias unsupported")
                    tmp = pool.tile(shape, f32, tag=tag)
                    nc.vector.tensor_scalar(tmp[:], in_ap, LEAKY, None, OP.mult)
                    nc.vector.tensor_tensor(out=out_ap, in0=in_ap, in1=tmp[:], op=OP.max)
                else:
                    nc.scalar.activation(out_ap, in_ap, AF.Prelu, alpha=LEAKY, bias=bias)

            def sim_bias_add(out_ap, in_ap, bias_col):
                # sim_safe fallback: add per-partition bias on DVE
                nc.vector.tensor_scalar(out_ap, in_ap, bias_col, None, OP.add)

            # ================= Phase A =================
            with (
                tc.tile_pool(name="pha", bufs=3) as pa,
                tc.tile_pool(name="pha_ps", bufs=2, space="PSUM") as pap,
            ):
                def slab(srcT, col0, to_tab, a2_k):
                    sl = pa.tile([D_IN, SLAB], f32, tag="sl")
                    nc.sync.dma_start(sl[:], srcT[:, col0:col0 + SLAB])
                    h1T_ps = pap.tile([D_OUT, SLAB], f32, tag="h1T_ps")
                    nc.tensor.matmul(h1T_ps[:], lhsT=W1T[:], rhs=sl[:], start=True, stop=True)
                    h1T = pa.tile([D_OUT, SLAB], f32, tag="h1T")
                    if cfg.sim_safe:
                        sim_bias_add(h1T[:], h1T_ps[:], b1c[:])
                        prelu(h1T[:], h1T[:], [D_OUT, SLAB], pa, "lrA")
                    else:
                        nc.scalar.activation(h1T[:], h1T_ps[:], AF.Prelu, alpha=LEAKY, bias=b1c[:])
                    hb_ps = pap.tile([128, SLAB], f32, tag="hb_ps")
                    for j in range(SLAB // 128):
                        nc.tensor.transpose(hb_ps[:, j * 128:(j + 1) * 128], h1T[:, j * 128:(j + 1) * 128], idn[:])
                    if to_tab:
                        p2T_ps = pap.tile([D_OUT, SLAB], f32, tag="p2T_ps")
                        nc.tensor.matmul(p2T_ps[:], lhsT=W2aT[:], rhs=sl[:], start=True, stop=True)
                        p2T = pa.tile([D_OUT, SLAB], f32, tag="p2T")
                        if cfg.sim_safe:
                            sim_bias_add(p2T[:], p2T_ps[:], b2c[:])
                        else:
                            nc.scalar.activation(p2T[:], p2T_ps[:], AF.Identity, bias=b2c[:])
                        pb_ps = pap.tile([128, SLAB], f32, tag="pb_ps")
                        for j in range(SLAB // 128):
                            nc.tensor.transpose(pb_ps[:, j * 128:(j + 1) * 128], p2T[:, j * 128:(j + 1) * 128], idn[:])
                        hb = pa.tile([128, SLAB // 128, 128], f32, tag="hb")
                        nc.scalar.activation(hb[:].rearrange("p a b -> p (a b)"), hb_ps[:], AF.Copy)
                        pb = pa.tile([128, SLAB // 128, 128], f32, tag="pb")
                        nc.vector.tensor_copy(pb[:].rearrange("p a b -> p (a b)"), pb_ps[:])
                        gslab = gtab[col0:col0 + SLAB, :].rearrange("(a p) b -> p a b", p=128)
                        nc.gpsimd.dma_start(gslab[:, :, D_OUT:], hb[:])
                        nc.gpsimd.dma_start(gslab[:, :, :D_OUT], pb[:])
                    else:
                        t0 = a2_k * (SLAB // 128)
                        nc.scalar.activation(
                            h1_my[:, t0:t0 + SLAB // 128, :].rearrange("p a b -> p (a b)"),
                            hb_ps[:], AF.Copy)
                        nc.gpsimd.dma_start(
                            h1_out[col0:col0 + SLAB, :].rearrange("(a p) b -> p a b", p=128),
                            h1_my[:, t0:t0 + SLAB // 128, :])
                        qscr = pa.tile([128, SLAB // 128, 128], f32, tag="qscr")
                        nc.vector.tensor_tensor(
                            out=qscr[:], in0=h1_my[:, t0:t0 + SLAB // 128, :],
                            in1=wdbc[:, None, :].to_broadcast([128, SLAB // 128, 128]), op=OP.mult)
                        nc.vector.tensor_reduce(
                            out=q_my[:, t0:t0 + SLAB // 128], in_=qscr[:], axis=X, op=OP.add)

                for k in range(K1):
                    slab(atomT_pad, k * SLAB, True, None)
                for k in range(KA2):
                    slab(atom_myT, k * SLAB, False, k)

            tc.strict_bb_all_engine_barrier()

            # ================= Phase B =================
            with (
                tc.tile_pool(name="phb", bufs=2) as pb_,
                tc.tile_pool(name="phb_nn_ps", bufs=1, space="PSUM") as pbn,
                tc.tile_pool(name="phb_ps", bufs=1, space="PSUM") as pbp,
            ):
                for s in range(S):
                    idx_t = pb_.tile([128, DEG], i32, tag="idx")
                    nc.sync.dma_start(idx_t[:], src_nat[s])
                    bT = pb_.tile([D_BOND, DEG, 128], f32, tag="bT")
                    nc.sync.dma_start(bT[:].rearrange("p a b -> p (a b)"), bondT_in[s])

                    nn_ps = pbn.tile([128, DEG * 128], f32, tag="nn_ps")
                    for q in range(DEG):
                        nc.tensor.matmul(
                            nn_ps[:, q * 128:(q + 1) * 128],
                            lhsT=bT[:, q, :], rhs=W2bT[:], start=True, stop=True)
                    gat = pb_.tile([128, DEG, 2 * D_OUT], f32, tag="gat")
                    for q in range(DEG):
                        nc.gpsimd.indirect_dma_start(
                            out=gat[:, q, :], out_offset=None,
                            in_=gtab[:],
                            in_offset=bass.IndirectOffsetOnAxis(ap=idx_t[:, q:q + 1], axis=0))
                    h1g = gat[:, :, D_OUT:]
                    pre = pb_.tile([128, DEG, 128], f32, tag="pre")
                    nc.vector.tensor_tensor(
                        out=pre[:], in0=nn_ps[:].rearrange("p (a b) -> p a b", a=DEG),
                        in1=gat[:, :, :D_OUT], op=OP.add)

                    nnact = pb_.tile([128, DEG * 128], f32, tag="nnact")
                    prelu(nnact[:], pre[:].rearrange("p a b -> p (a b)"), [128, DEG * 128], pb_, "lrB")
                    nnw = pb_.tile([128, DEG, 128], f32, tag="nnw")
                    nc.vector.tensor_tensor(
                        out=nnw[:], in0=nnact[:].rearrange("p (a b) -> p a b", a=DEG),
                        in1=webc[:, None, :].to_broadcast([128, DEG, 128]), op=OP.mult)
                    pl = pb_.tile([128, DEG], f32, tag="pl")
                    nc.vector.tensor_reduce(out=pl[:], in_=nnw[:], axis=X, op=OP.add)

                    z8 = pb_.tile([128, DEG], f32, tag="z8")
                    nc.vector.tensor_scalar(z8[:], pl[:], q_my[:, s:s + 1], _BW[0], OP.add, OP.add)
                    l8 = pb_.tile([128, DEG], f32, tag="l8")
                    prelu(l8[:], z8[:], [128, DEG], pb_, "lrC")
                    w8 = pb_.tile([128, DEG], f32, tag="w8")
                    dsum = pb_.tile([128, 1], f32, tag="dsum")
                    nc.scalar.activation(w8[:], l8[:], AF.Exp, accum_out=dsum[:])
                    rd = pb_.tile([128, 1], f32, tag="rd")
                    nc.vector.reciprocal(rd[:], dsum[:])
                    sc8 = pb_.tile([128, DEG], f32, tag="sc8")
                    nc.vector.tensor_scalar(sc8[:], w8[:], rd[:], None, OP.mult)

                    wh = pb_.tile([128, DEG, 128], f32, tag="wh")
                    nc.vector.tensor_tensor(
                        out=wh[:], in0=h1g,
                        in1=sc8[:, :, None].to_broadcast([128, DEG, 128]), op=OP.mult)
                    traw = pb_.tile([128, 128], f32, tag="traw")
                    nc.vector.tensor_reduce(
                        out=traw[:], in_=wh[:].rearrange("p a b -> p b a"), axis=X, op=OP.add)

                    trT_ps = pbp.tile([128, 128], f32, tag="tps")
                    nc.tensor.transpose(trT_ps[:], traw[:], idn[:])
                    trT = pb_.tile([128, 128], f32, tag="trT")
                    nc.scalar.activation(trT[:], trT_ps[:], AF.Copy)
                    wa_ps = pbp.tile([128, D_OUT], f32, tag="wa_ps")
                    nc.tensor.matmul(wa_ps[:], lhsT=trT[:], rhs=WaT[:], start=True, stop=False)
                    nc.tensor.matmul(wa_ps[:], lhsT=onesr[:], rhs=bar[:], start=False, stop=True)

                    # elu(x) + 1 = relu(x) + exp(min(x, 0)); the -1 is folded into bg
                    m0 = pb_.tile([128, 128], f32, tag="m0")
                    nc.vector.tensor_scalar(m0[:], wa_ps[:], 0.0, None, OP.min)
                    e0 = pb_.tile([128, 128], f32, tag="e0")
                    nc.scalar.activation(e0[:], m0[:], AF.Exp)
                    r0 = pb_.tile([128, 128], f32, tag="r0")
                    nc.vector.tensor_scalar(r0[:], wa_ps[:], 0.0, None, OP.max)
                    ctxs = pb_.tile([128, 128], f32, tag="ctxs")
                    nc.vector.tensor_tensor(out=ctxs[:], in0=r0[:], in1=e0[:], op=OP.add)

                    cT_ps = pbp.tile([128, 128], f32, tag="tps")
                    nc.tensor.transpose(cT_ps[:], ctxs[:], idn[:])
                    cT = pb_.tile([128, 128], f32, tag="cT")
                    nc.scalar.activation(cT[:], cT_ps[:], AF.Copy)
                    hT_ps = pbp.tile([128, 128], f32, tag="tps")
                    nc.tensor.transpose(hT_ps[:], h1_my[:, s, :], idn[:])
                    hT = pb_.tile([128, 128], f32, tag="hT")
                    nc.scalar.activation(hT[:], hT_ps[:], AF.Copy)

                    g_ps = pbp.tile([128, 3 * D_OUT], f32, tag="g_ps")
                    nc.tensor.matmul(g_ps[:], lhsT=cT[:], rhs=WihT[:], start=True, stop=False)
                    nc.tensor.matmul(g_ps[:, :2 * D_OUT], lhsT=hT[:], rhs=WhrzT[:], start=False, stop=False)
                    nc.tensor.matmul(g_ps[:], lhsT=onesr[:], rhs=bgr[:], start=False, stop=True)
                    hn_ps = pbp.tile([128, D_OUT], f32, tag="hn_ps")
                    nc.tensor.matmul(hn_ps[:], lhsT=hT[:], rhs=WhnT[:], start=True, stop=False)
                    nc.tensor.matmul(hn_ps[:], lhsT=onesr[:], rhs=bhnr[:], start=False, stop=True)

                    # sigmoid(x) = 0.5 * (1 + tanh(x/2)) -- keeps ACT on one table set
                    rzt = pb_.tile([128, 2 * D_OUT], f32, tag="rzt")
                    nc.scalar.activation(rzt[:], g_ps[:, :2 * D_OUT], AF.Tanh, scale=0.5)
                    rz = pb_.tile([128, 2 * D_OUT], f32, tag="rz")
                    nc.vector.tensor_scalar(rz[:], rzt[:], 0.5, 0.5, OP.mult, OP.add)
                    rhn = pb_.tile([128, 128], f32, tag="rhn")
                    nc.vector.tensor_tensor(out=rhn[:], in0=hn_ps[:], in1=rz[:, :128], op=OP.mult)
                    npre = pb_.tile([128, 128], f32, tag="npre")
                    nc.vector.tensor_tensor(out=npre[:], in0=rhn[:], in1=g_ps[:, 2 * D_OUT:], op=OP.add)
                    ngate = pb_.tile([128, 128], f32, tag="ngate")
                    nc.scalar.activation(ngate[:], npre[:], AF.Tanh)
                    d1 = pb_.tile([128, 128], f32, tag="d1")
                    nc.vector.tensor_tensor(out=d1[:], in0=h1_my[:, s, :], in1=ngate[:], op=OP.subtract)
                    d2 = pb_.tile([128, 128], f32, tag="d2")
                    nc.vector.tensor_tensor(out=d2[:], in0=d1[:], in1=rz[:, 128:], op=OP.mult)
                    fin = pb_.tile([128, 128], f32, tag="fin")
                    nc.vector.tensor_tensor(out=fin[:], in0=d2[:], in1=ngate[:], op=OP.add)
                    nc.sync.dma_start(final_out[s * 128:(s + 1) * 128, :], fin[:])

    nc.finalize()
    return nc


_BW = [0.0]


def _host_prep(inputs, cfg):
    atom = np.ascontiguousarray(inputs["atom_features"], dtype=np.float32)
    bond = np.ascontiguousarray(inputs["bond_feats"], dtype=np.float32)
    src = np.ascontiguousarray(inputs["src"], dtype=np.int32)
    W1 = inputs["W1"].astype(np.float32)
    b1 = inputs["b1"].astype(np.float32)
    W2 = inputs["W2"].astype(np.float32)
    b2 = inputs["b2"].astype(np.float32)
    Wa = inputs["Wa"].astype(np.float32)
    ba = inputs["ba"].astype(np.float32)
    Ww = inputs["Ww"].astype(np.float32)
    bw = inputs["bw"].astype(np.float32)
    W_ih = inputs["W_ih"].astype(np.float32)
    b_ih = inputs["b_ih"].astype(np.float32)
    W_hh = inputs["W_hh"].astype(np.float32)
    b_hh = inputs["b_hh"].astype(np.float32)

    n, C = cfg.n_nodes, cfg.n_cores
    ncn, nce = cfg.nc_nodes, cfg.nc_edges
    S, EPAD, NTAB, NPAD2 = cfg.S, cfg.epad, cfg.ntab, cfg.npad2

    atomT_pad = np.zeros((D_IN, NTAB), np.float32)
    atomT_pad[:, :n] = atom.T

    w_d = Ww[0, :D_OUT].copy()
    w_e = Ww[0, D_OUT:].copy()
    _BW[0] = float(bw[0])

    shared = {
        "atomT_pad": atomT_pad,
        "W1T_in": np.ascontiguousarray(W1.T),
        "W2aT_in": np.ascontiguousarray(W2[:, :D_IN].T),
        "W2bT_in": np.ascontiguousarray(W2[:, D_IN:].T),
        "WaT_in": np.ascontiguousarray(Wa.T),
        "WihT_in": np.ascontiguousarray(W_ih.T),
        "WhrzT_in": np.ascontiguousarray(W_hh[:2 * D_OUT].T),
        "WhnT_in": np.ascontiguousarray(W_hh[2 * D_OUT:].T),
        "webc_in": np.ascontiguousarray(np.tile(w_e[None, :], (128, 1))),
        "wdbc_in": np.ascontiguousarray(np.tile(w_d[None, :], (128, 1))),
        "b1c_in": b1[:, None].copy(),
        "b2c_in": b2[:, None].copy(),
        "ba_in": ba[None, :].copy(),
        "bg_in": (b_ih - W_ih.sum(axis=1)
                  + np.concatenate([b_hh[:2 * D_OUT], np.zeros(D_OUT, np.float32)])
                  )[None, :].astype(np.float32),
        "bhn_in": b_hh[2 * D_OUT:][None, :].copy(),
        "ones_in": np.ones((1, 128), np.float32),
    }

    in_maps = []
    for c in range(C):
        aT = np.zeros((D_IN, NPAD2), np.float32)
        aT[:, :ncn] = atom[c * ncn:(c + 1) * ncn].T
        src_pad = np.zeros(EPAD, np.int32)
        src_pad[:nce] = src[c * nce:(c + 1) * nce]
        bond_pad = np.zeros((EPAD, D_BOND), np.float32)
        bond_pad[:nce] = bond[c * nce:(c + 1) * nce]
        src_nat = np.ascontiguousarray(src_pad.reshape(S, 128, DEG))
        bondT = np.ascontiguousarray(
            bond_pad.reshape(S, 128, DEG, D_BOND).transpose(0, 3, 2, 1)
        ).reshape(S, D_BOND, 1024)
        im = dict(shared)
        im["atom_myT"] = aT
        im["src_nat"] = src_nat
        im["bondT_in"] = bondT
        in_maps.append(im)
    return in_maps


def kernel(**inputs):
    from concourse.bass_utils import run_bass_kernel_spmd

    cfg = _Cfg()
    in_maps = _host_prep(inputs, cfg)
    key = (cfg.n_nodes, cfg.n_cores)
    if key not in _cache:
        _cache[key] = _build(cfg)
    nc = _cache[key]
    res = run_bass_kernel_spmd(nc, in_maps, list(range(cfg.n_cores)))
    ncn = cfg.nc_nodes
    final = np.concatenate([res.results[c]["final_out"][:ncn] for c in range(cfg.n_cores)], axis=0)
    h1 = np.concatenate([res.results[c]["h1_out"][:ncn] for c in range(cfg.n_cores)], axis=0)
    return final, h1
